# revision 27
# baseline (speedup 1.0000x reference)
"""Bahdanau attention scorer on 8 NeuronCores (Trainium2, Bass/Tile).

scores[t,b,s] = sum_a v_a[a] * tanh( (W_s @ enc[s,b])[a] + (W_t @ dec[t,b])[a] + b_t[a] )

Algorithm: Fourier factorization of tanh. tanh(x) ~= sum_m b_m sin(om_m x),
and sin(om(e+d)) = sin(om e)cos(om d) + cos(om e)sin(om d) separates the
(src x trg x att) elementwise tensor into 2M rank-1 products, turning the
score into 2M PE matmuls over small per-mode feature maps of E and D.

Sharding: data-parallel over batch (32 -> 4 per core); params replicated.

Per-core dataflow (BC=4 batch elems):
  - Host pre-packs enc/dec/W into fp16 "SBUF-image" layouts ([p, hb, b, s])
    so each input tensor lands in 2 DMAs with 8KB-contiguous descriptors.
  - PE projects E = W^T-stationary @ enc -> [a=128, s] PSUM f32; DVE stages
    to fp16 (D gets +b_t folded into the staging add).
  - Phases: k = round(x*om/2pi*4096) as int16 (tensor_scalar mult, +1024
    for the cos phase), masked to [0,4096) with bitwise_and; ACT reads the
    int16 directly: sin(2pi*k/4096 - pi) = -sin(x) (sign cancels in the
    products). sin+cos phases are packed per (tensor, mode-chunk) so each
    ACT call is one big op; chunks pipeline DVE -> ACT -> PE.
  - DVE folds b_m*v_a into the E-side features; PE accumulates
    psum[t,s] += cosD_m^T @ (bv sinE_m) + sinD_m^T @ (bv cosE_m), one
    PSUM bank per (b, t-half), split into per-chunk contiguous bursts.
  - DVE copies psum -> SBUF f32, one DMA per b writes [t-half, b, s].
"""

import math
import os

import numpy as np

SRC, TRG, BATCH, HID, ATT = 256, 256, 32, 512, 128
N_CORES = 8
BC = BATCH // N_CORES  # batch elems per core
NHB = HID // 128

_FITS = {
    4: (
        [0.24048614356302503, 0.7142878949351071, 1.2636497589697908,
         2.192765146810882],
        [1.2561088928144657, 0.3321445440522845, 0.19982653245030885,
         0.07227558569142113],
    ),
    5: (
        [0.23774, 0.7174, 1.19121, 1.75033, 2.68315],
        [1.24227, 0.35281, 0.14575, 0.09248, 0.03351],
    ),
    6: (
        [0.23577199575936503, 0.7094372171270281, 1.1918260670145222,
         1.669460403711867, 2.23579441296016, 3.17224131311109],
        [1.246465601651194, 0.34942946437961786, 0.15444004207681808,
         0.06848353196928972, 0.04330376154947291, 0.015576992892809388],
    ),
    8: (
        [0.23212375121156203, 0.6986266119227419, 1.1712597614035027,
         1.6515962407531752, 2.143424311855918, 2.6303732287478443,
         3.2081300864284477, 4.150433048550138],
        [1.2465540672138469, 0.3522442065700904, 0.15538545598785888,
         0.07270679270906916, 0.03446026366388061, 0.015447426550680666,
         0.009498141278776302, 0.0033631766831473927],
    ),
}
M_MODES = int(os.environ.get("K_M", "4"))
OMEGA, B_COEF = _FITS[M_MODES]
GRID = 4096
# mode chunks: big first chunk, small last chunk to shrink the PE tail
_TAIL = int(os.environ.get("K_TAIL", "2"))
CHUNKS = [(0, M_MODES - _TAIL), (M_MODES - _TAIL, M_MODES)] if _TAIL else [(0, M_MODES)]

_NC_CACHE = {}


def build_nc():
    import concourse.tile as tile
    from concourse import bacc, mybir

    f32 = mybir.dt.float32
    f16 = mybir.dt.float16
    i16 = mybir.dt.int16
    SIN = mybir.ActivationFunctionType.Sin
    MULT = mybir.AluOpType.mult
    ADD = mybir.AluOpType.add
    AND = mybir.AluOpType.bitwise_and

    SB = BC * SRC  # 1024
    M = M_MODES
    CHMAX = max(hi - lo for lo, hi in CHUNKS)

    # packed phase/trig column offsets: chunk j holds [sin modes | cos modes]
    def col_s(m):
        for lo, hi in CHUNKS:
            if lo <= m < hi:
                return 2 * lo * SB + (m - lo) * SB
        raise ValueError(m)

    def col_c(m):
        for lo, hi in CHUNKS:
            if lo <= m < hi:
                return 2 * lo * SB + (hi - lo) * SB + (m - lo) * SB
        raise ValueError(m)

    nc = bacc.Bacc(
        "TRN2", target_bir_lowering=False, debug=False, num_devices=N_CORES
    )
    enc_in = nc.dram_tensor("enc_img", [128, NHB, BC, SRC], f16, kind="ExternalInput")
    dec_in = nc.dram_tensor("dec_img", [128, NHB, BC, TRG], f16, kind="ExternalInput")
    ws_in = nc.dram_tensor("ws_img", [128, NHB, ATT], f16, kind="ExternalInput")
    wt_in = nc.dram_tensor("wt_img", [128, NHB, ATT], f16, kind="ExternalInput")
    bt_in = nc.dram_tensor("b_t", [ATT, 1], f32, kind="ExternalInput")
    bv_in = nc.dram_tensor("bv", [ATT, M_MODES], f32, kind="ExternalInput")
    out = nc.dram_tensor("scores", [TRG, BC, SRC], f32, kind="ExternalOutput")

    with tile.TileContext(nc) as tc:
        with (
            tc.tile_pool(name="consts", bufs=1) as consts,
            tc.tile_pool(name="raw", bufs=1) as raw_pool,
            tc.tile_pool(name="ed", bufs=1) as ed_pool,
            tc.tile_pool(name="feat", bufs=1) as feat_pool,
            tc.tile_pool(name="scr", bufs=1) as scr_pool,
            tc.tile_pool(name="trig", bufs=1) as trig_pool,
            tc.tile_pool(name="ve", bufs=1) as ve_pool,
            tc.tile_pool(name="osb", bufs=1) as osb_pool,
            tc.tile_pool(name="proj_ps", bufs=2, space="PSUM") as proj_ps,
            tc.tile_pool(name="sc_ps", bufs=BC, space="PSUM") as sc_ps,
        ):
            wsT_sb = consts.tile([128, NHB, 128], f16)
            wtT_sb = consts.tile([128, NHB, 128], f16)
            bt_sb = consts.tile([128, 1], f32)
            bv_sb = consts.tile([128, M], f32)
            negpi = consts.tile([128, 1], f32)
            nc.vector.memset(negpi[:], -math.pi)
            warm = consts.tile([1, 2], f32)
            nc.vector.memset(warm[:], 0.0)
            nc.scalar.activation(warm[:], warm[:], SIN)

            # inputs first on the queue (big transfers), then small consts
            enc_sb = raw_pool.tile([128, NHB, BC, SRC], f16, tag="enc", name="enc_sb")
            dec_sb = raw_pool.tile([128, NHB, BC, TRG], f16, tag="dec", name="dec_sb")
            H2 = NHB // 2
            nc.sync.dma_start(out=enc_sb[:, 0:H2], in_=enc_in[:, 0:H2])
            nc.sync.dma_start(out=wsT_sb[:], in_=ws_in[:])
            nc.sync.dma_start(out=dec_sb[:, 0:H2], in_=dec_in[:, 0:H2])
            nc.sync.dma_start(out=wtT_sb[:], in_=wt_in[:])
            nc.sync.dma_start(out=enc_sb[:, H2:NHB], in_=enc_in[:, H2:NHB])
            nc.sync.dma_start(out=bt_sb[:], in_=bt_in[:])
            nc.sync.dma_start(out=dec_sb[:, H2:NHB], in_=dec_in[:, H2:NHB])
            nc.sync.dma_start(out=bv_sb[:], in_=bv_in[:])

            # --- projection: E/D [a=128, (b,s)=SB] fp16 ---
            ED = ed_pool.tile([128, 2, SB], f16, tag="ED", name="ED")

            def proj(name, x_sb, wT, ti):
                # two batch elems per matmul (rhs cols are b-adjacent in the
                # SBUF image): N=512 streams halve LDWEIGHTS + instruction
                # count on the front critical path
                for bp in range(BC // 2):
                    pps = proj_ps.tile([128, 512], f32, tag="proj", name=f"pp_{name}{bp}")
                    for hb in range(NHB):
                        nc.tensor.matmul(
                            pps[:],
                            wT[:, hb, :],
                            x_sb[:, hb, 2 * bp : 2 * bp + 2, :],
                            start=(hb == 0),
                            stop=(hb == NHB - 1),
                        )
                    cols = slice(2 * bp * 256, (2 * bp + 2) * 256)
                    if name == "e":
                        nc.vector.tensor_copy(ED[:, ti, cols], pps[:])
                    else:
                        nc.vector.tensor_scalar_add(ED[:, ti, cols], pps[:], bt_sb[:, 0:1])

            # packed tiles
            kE = feat_pool.tile([128, 2 * M * SB], i16, tag="kE", name="kE")
            kD = feat_pool.tile([128, 2 * M * SB], i16, tag="kD", name="kD")
            trE = trig_pool.tile([128, 2 * M * SB], f16, tag="trE", name="trE")
            trD = trig_pool.tile([128, 2 * M * SB], f16, tag="trD", name="trD")
            scrA = scr_pool.tile([128, 2, 2 * CHMAX * SB], i16, tag="scrA", name="scrA")
            veA = ve_pool.tile([128, 2, M * SB], f16, tag="veA", name="veA")

            sc_tiles = {}
            for b in range(BC):
                for th in range(TRG // 128):
                    sc_tiles[(b, th)] = sc_ps.tile(
                        [128, 256], f32, tag="sc", name=f"sc{b}_{th}"
                    )

            def feats(ti, k_t, lo, hi):
                # scratch laid out exactly like the packed chunk
                # ([sin modes | cos modes]) so ONE wide AND masks everything
                ch = hi - lo
                for m in range(lo, hi):
                    c = OMEGA[m] / (2 * math.pi) * GRID
                    i = m - lo
                    nc.vector.tensor_scalar(
                        scrA[:, ti, i * SB : (i + 1) * SB], ED[:, ti, :], c, None, MULT
                    )
                    nc.vector.tensor_scalar(
                        scrA[:, ti, (ch + i) * SB : (ch + i + 1) * SB],
                        ED[:, ti, :], c, GRID // 4, MULT, ADD,
                    )
                nc.vector.tensor_scalar(
                    k_t[:, 2 * lo * SB : 2 * hi * SB],
                    scrA[:, ti, 0 : 2 * ch * SB],
                    GRID - 1, None, AND,
                )

            def act(k_t, tr, lo, hi):
                cc = slice(2 * lo * SB, 2 * hi * SB)
                nc.scalar.activation(
                    tr[:, cc], k_t[:, cc], SIN,
                    bias=negpi[:, 0:1], scale=2 * math.pi / GRID,
                )

            def vfolds(lo, hi):
                for m in range(lo, hi):
                    mc = slice(m * SB, (m + 1) * SB)
                    nc.vector.tensor_scalar(
                        veA[:, 0, mc], trE[:, col_s(m) : col_s(m) + SB],
                        bv_sb[:, m : m + 1], None, MULT,
                    )
                    nc.vector.tensor_scalar(
                        veA[:, 1, mc], trE[:, col_c(m) : col_c(m) + SB],
                        bv_sb[:, m : m + 1], None, MULT,
                    )

            def burst(lo, hi):
                for b in range(BC):
                    for th in range(TRG // 128):
                        sc_t = sc_tiles[(b, th)]
                        for m in range(lo, hi):
                            lc_c = col_c(m) + b * 256 + th * 128
                            lc_s = col_s(m) + b * 256 + th * 128
                            rc = slice(m * SB + b * 256, m * SB + (b + 1) * 256)
                            nc.tensor.matmul(
                                sc_t[:], trD[:, lc_c : lc_c + 128], veA[:, 0, rc],
                                start=(m == 0), stop=False,
                            )
                            nc.tensor.matmul(
                                sc_t[:], trD[:, lc_s : lc_s + 128], veA[:, 1, rc],
                                start=False, stop=(m == M - 1),
                            )

            # staged emission: per-engine queue order is what matters.
            # E's pipeline runs ahead so ACT starts as early as possible and
            # the vfolds (DVE, post-E-ACT) hide under D's ACT.
            (lo1, hi1), (lo2, hi2) = CHUNKS if len(CHUNKS) == 2 else (CHUNKS[0], CHUNKS[0])
            proj("e", enc_sb, wsT_sb, 0)
            feats(0, kE, lo1, hi1)
            act(kE, trE, lo1, hi1)
            proj("d", dec_sb, wtT_sb, 1)
            feats(1, kD, lo1, hi1)
            act(kD, trD, lo1, hi1)
            if len(CHUNKS) == 2:
                feats(0, kE, lo2, hi2)
                vfolds(lo1, hi1)
                burst(lo1, hi1)
                act(kE, trE, lo2, hi2)
                feats(1, kD, lo2, hi2)
                act(kD, trD, lo2, hi2)
                vfolds(lo2, hi2)
                burst(lo2, hi2)
            else:
                vfolds(lo1, hi1)
                burst(lo1, hi1)

            # --- drain: psum -> sbuf f32, one DMA per b ---
            otA = osb_pool.tile([128, BC, 2, 256], f32, tag="otA", name="otA")
            for b in range(BC):
                for th in range(TRG // 128):
                    nc.vector.tensor_copy(otA[:, b, th, :], sc_tiles[(b, th)][:])
                nc.sync.dma_start(
                    out=out[:, b, :].rearrange("(th p) s -> p th s", p=128),
                    in_=otA[:, b],
                )
    nc.compile()
    return nc


def _get_nc():
    if "nc" not in _NC_CACHE:
        _NC_CACHE["nc"] = build_nc()
    return _NC_CACHE["nc"]


def _prep_in_maps(inputs):
    f16 = np.float16
    dec_out = np.asarray(inputs["dec_out"], dtype=np.float32)
    enc_outs = np.asarray(inputs["enc_outs"], dtype=np.float32)
    ws_img = np.ascontiguousarray(
        np.asarray(inputs["W_s"], dtype=np.float32)
        .T.reshape(NHB, 128, ATT)
        .transpose(1, 0, 2)
        .astype(f16)
    )
    wt_img = np.ascontiguousarray(
        np.asarray(inputs["W_t"], dtype=np.float32)
        .T.reshape(NHB, 128, ATT)
        .transpose(1, 0, 2)
        .astype(f16)
    )
    b_t = np.asarray(inputs["b_t"], dtype=np.float32).reshape(ATT, 1)
    v_a = np.asarray(inputs["v_a"], dtype=np.float32).reshape(ATT, 1)
    bv = np.ascontiguousarray(
        v_a * np.asarray(B_COEF, dtype=np.float32)[None, :]
    ).astype(np.float32)

    in_maps = []
    for c in range(N_CORES):
        bsl = slice(c * BC, (c + 1) * BC)
        enc_img = np.ascontiguousarray(
            enc_outs[:, bsl, :]
            .transpose(2, 1, 0)
            .reshape(NHB, 128, BC, SRC)
            .transpose(1, 0, 2, 3)
            .astype(f16)
        )
        dec_img = np.ascontiguousarray(
            dec_out[:, bsl, :]
            .transpose(2, 1, 0)
            .reshape(NHB, 128, BC, TRG)
            .transpose(1, 0, 2, 3)
            .astype(f16)
        )
        in_maps.append(
            {
                "enc_img": enc_img,
                "dec_img": dec_img,
                "ws_img": ws_img,
                "wt_img": wt_img,
                "b_t": b_t,
                "bv": bv,
            }
        )
    return in_maps


def kernel(dec_out, enc_outs, W_s, W_t, b_t, v_a):
    from concourse.bass_utils import run_bass_kernel_spmd

    nc = _get_nc()
    in_maps = _prep_in_maps(
        {
            "dec_out": dec_out,
            "enc_outs": enc_outs,
            "W_s": W_s,
            "W_t": W_t,
            "b_t": b_t,
            "v_a": v_a,
        }
    )
    res = run_bass_kernel_spmd(nc, in_maps, list(range(N_CORES)))
    return np.concatenate([r["scores"] for r in res.results], axis=1)
